# revision 2
# baseline (speedup 1.0000x reference)
"""Stage-4 Trainium2 Bass kernel for nn_BilinearFullSymLoss.

Math (same derivation as the stage-3 kernel, bc re-anchored to unshifted rows):
  delta(i,j) = wA0*G(i,j) + wA1*G(i+1,j) + wB0*bc(i+rb,j) + wB1*bc(i+rb+1,j)
  bc(i,j)    = cb0*G(i, j+cb) + cb1*G(i, j+cb+1)    (col interp, same row)
  pos: wA=(1,0),         wB=(-(1-fy),-fy), rb=dy1,   cb=dx1
  neg: wA=(-fy,-(1-fy)), wB=(1,0),         rb=dy1+1, cb=dx1
       (neg evaluated at j' = j-dx1; host sums columns [-dx1, W))
  loss = m^2 * sum(valid delta^2) / (rows*cols); host does the scalar math.

Device plan per core (4 samples), no DRAM round-trip:
- one DMA per sample brings both channels (the last sample's load is split
  in three so its compute overlaps the transfer tail)
- ACT does a*g0; DVE scalar_tensor_tensor fuses G = b*g1 + (a*g0) into a
  per-q-slotted fp16 SBUF tile (each 512-col block padded by 8+8 zero
  cols, so dynamic column reads never leave their slot -> race-free)
- the dynamic column shift j+cb is a register-offset (values_load + reg
  adds) DVE read of that tile: t5 = cb1*G(+cb+1), t6 = cb0*G(+cb),
  bc = t5 + t6 (tensor_scalar gets the 4x fp16 DVE mode; STT does not)
- the row shift rb lives in the host-built banded lhsT matrices (band
  offsets {rb, rb+1}); cross-seam matrices carry the band across the
  128-row partition blocks; for the last block the missing cross rows are
  provably masked (neg has wB1=0)
- PE accumulates delta in PSUM; ACT squares per block; PE does iv-masked
  column sums; ACT copies the [1,W] result to SBUF; one final SP DMA.
  All compute is split in half-sample units so the post-last-DMA tail is
  short.  Engine routing keeps waits off the SP sequencer that issues
  input loads.
"""

import sys

sys.path.insert(0, "/opt/trn_rl_repo")

import numpy as np

import concourse.bass as bass
import concourse.tile as tile
from concourse import mybir
from concourse.bass_utils import run_bass_kernel_spmd

H = 512
W = 512
P = 128
Q = H // P
NS = 4
NCORES = 8
PADL = 8
PADR = 8
SS = PADL + W + PADR      # 528: slot size per q block
GF = Q * SS               # 2112: per-buffer free size of padded G tile
NMAT = NS * 2 * P         # mats region width in fmats
NIV = NS * Q
NXTR = 2 * P              # s3's PE-direct main matrices: Bc0, Bc1
KX = 8                    # cross matrices only have rows k < rb+1 <= 7
FM = NMAT + NIV + NXTR    # fmats cols; crosses live in the small xm tensor
XW = NMAT + 2 * P         # xm cols: xA/xB per sample + s3's xBc0/xBc1
NPF = 4                   # a, b, cb0, cb1
PFW = NS * NPF + NS + 2   # scalars + bitcast i32 DVE offsets + 2 PE offsets

F32 = mybir.dt.float32
F16 = mybir.dt.float16
I32 = mybir.dt.int32

COL_A, COL_B, COL_CB0, COL_CB1 = range(NPF)

_CACHE = {}


def _split_multiwaits(nc):
    """The staged walrus accepts one sync wait per instruction; hoist extras
    onto single-wait NoOps."""
    n = 0
    for fn in nc.m.functions:
        for bb in fn.blocks:
            newlist = []
            for ins in bb.instructions:
                si = ins.sync_info
                if si is not None and si.on_wait is not None and len(si.on_wait) > 1:
                    waits = list(si.on_wait)
                    for w in waits[:-1]:
                        n += 1
                        newlist.append(mybir.InstNoOp(
                            name=f"WSPLIT-{n}-{ins.name}", opcode="NoOp",
                            engine=ins.engine,
                            sync_info=mybir.SyncInfo(on_wait=[w], on_update=[])))
                    ins.sync_info = mybir.SyncInfo(
                        on_wait=[waits[-1]], on_update=list(si.on_update))
                newlist.append(ins)
            bb.instructions = newlist
    return n


def _build_program():
    nc = bass.Bass("TRN2", target_bir_lowering=False, debug=False)

    g = nc.dram_tensor("g", [NS, 2, H, W], F32, kind="ExternalInput")
    pfi = nc.dram_tensor("pfi", [P, PFW], F32, kind="ExternalInput")
    fmats = nc.dram_tensor("fmats", [P, FM], F16, kind="ExternalInput")
    xm = nc.dram_tensor("xm", [KX, XW], F16, kind="ExternalInput")
    out = nc.dram_tensor("out", [NS, W], F32, kind="ExternalOutput")

    with tile.TileContext(nc) as tc:
        with (
            tc.tile_pool(name="consts", bufs=1) as consts,
            tc.tile_pool(name="io", bufs=2) as io,
            tc.tile_pool(name="work", bufs=2) as work,
            tc.tile_pool(name="psd", bufs=2, space="PSUM") as psdp,
        ):
            pfsb = None
            fmsb = None
            xmsb = None
            gsb2 = None
            osb = None
            st = [dict() for _ in range(NS)]

            def pcol(s, c):
                return pfsb[:, s * NPF + c: s * NPF + c + 1]

            def emit_consts():
                nonlocal pfsb, fmsb, xmsb, gsb2, osb
                pfsb = consts.tile([P, PFW], F32)
                nc.sync.dma_start(pfsb[:], pfi[:])
                fmsb = consts.tile([P, FM], F16)
                nc.sync.dma_start(fmsb[:], fmats[:])
                xmsb = consts.tile([KX, XW], F16)
                nc.sync.dma_start(xmsb[:], xm[:])
                gsb2 = consts.tile([P, 2, GF], F16)
                nc.gpsimd.memset(gsb2[:], 0.0)
                osb = consts.tile([1, NS * W], F32)

            def emit_load_ch(s, ch):
                key = f"g{ch}t"
                if key not in st[s]:
                    st[s][key] = io.tile([P, Q, W], F32, tag=key,
                                         name=f"{key}_{s}")
                nc.sync.dma_start(
                    st[s][key][:],
                    bass.AP(tensor=g, offset=(s * 2 + ch) * H * W,
                            ap=[[W, P], [P * W, Q], [1, W]]))

            def emit_load_ch1_part(s, qlo, qn):
                key = "g1t"
                if key not in st[s]:
                    st[s][key] = io.tile([P, Q, W], F32, tag=key,
                                         name=f"{key}_{s}")
                nc.sync.dma_start(
                    st[s][key][:, qlo:qlo + qn, :],
                    bass.AP(tensor=g,
                            offset=(s * 2 + 1) * H * W + qlo * P * W,
                            ap=[[W, P], [P * W, qn], [1, W]]))

            def emit_m0(s, qlo, qn):
                if "m0" not in st[s]:
                    st[s]["m0"] = work.tile([P, Q, W], F16, tag="m0",
                                            name=f"m0_{s}")
                sl = slice(qlo, qlo + qn)
                nc.gpsimd.tensor_scalar(out=st[s]["m0"][:, sl, :],
                                        in0=st[s]["g0t"][:, sl, :],
                                        scalar1=pcol(s, COL_A), scalar2=None,
                                        op0=mybir.AluOpType.mult)

            def emit_vload(s):
                st[s]["off0"] = nc.values_load(
                    pfsb[0:1, NS * NPF + s:NS * NPF + s + 1].bitcast(I32),
                    engines=(mybir.EngineType.DVE,),
                    skip_runtime_bounds_check=True)

            def emit_comb(s, qlo, qn, eng=None):
                b = s & 1
                sl = slice(qlo, qlo + qn) if qn > 1 else qlo
                gap = ([[2 * GF, P], [SS, qn], [1, W]] if qn > 1
                       else [[2 * GF, P], [1, W]])
                gout = bass.AP(tensor=gsb2.tensor,
                               offset=b * GF + qlo * SS + PADL, ap=gap)
                (eng or nc.vector).scalar_tensor_tensor(
                    out=gout, in0=st[s]["g1t"][:, sl, :],
                    scalar=pcol(s, COL_B), in1=st[s]["m0"][:, sl, :],
                    op0=mybir.AluOpType.mult, op1=mybir.AluOpType.add)

            def emit_bc(s, qlo, qn):
                b = s & 1
                for nm in ("t5", "t6", "bc"):
                    if nm not in st[s]:
                        st[s][nm] = work.tile([P, Q, W], F16, tag=nm,
                                              name=f"{nm}_{s}")
                t5, t6, bc = st[s]["t5"], st[s]["t6"], st[s]["bc"]
                sl = slice(qlo, qlo + qn) if qn > 1 else qlo
                dep = b * GF + qlo * SS
                base = (st[s]["off0"] + (qlo * SS)) if qlo else st[s]["off0"]

                def dyn(off):
                    ap = ([[2 * GF, P], [SS, qn], [1, W]] if qn > 1
                          else [[2 * GF, P], [1, W]])
                    return bass.AP(tensor=gsb2.tensor, offset=off,
                                   ap=ap, dep_tracking_offset=dep)

                nc.vector.tensor_scalar(
                    out=t5[:, sl, :], in0=dyn(base + 1),
                    scalar1=pcol(s, COL_CB1), scalar2=None,
                    op0=mybir.AluOpType.mult)
                nc.vector.tensor_scalar(
                    out=t6[:, sl, :], in0=dyn(base),
                    scalar1=pcol(s, COL_CB0), scalar2=None,
                    op0=mybir.AluOpType.mult)
                nc.vector.tensor_tensor(
                    out=bc[:, sl, :], in0=t5[:, sl, :], in1=t6[:, sl, :],
                    op=mybir.AluOpType.add)

            def gq_ap(s, q):
                b = s & 1
                return bass.AP(tensor=gsb2.tensor,
                               offset=b * GF + q * SS + PADL,
                               ap=[[2 * GF, P], [1, W]])

            def psd_of(s):
                if "psd" not in st[s]:
                    st[s]["psd"] = psdp.tile([P, Q, W], F32, tag="psd",
                                             name=f"psd_{s}")
                return st[s]["psd"]

            def mm(s, q, lhsT, rhs, st_, sp):
                nc.tensor.matmul(psd_of(s)[:, q, :], lhsT=lhsT, rhs=rhs,
                                 start=st_, stop=sp, skip_group_check=True)

            def mats_of(s):
                mA = fmsb[:, (2 * s) * P:(2 * s + 1) * P]
                mB = fmsb[:, (2 * s + 1) * P:(2 * s + 2) * P]
                xA = xmsb[:, (2 * s) * P:(2 * s + 1) * P]
                xB = xmsb[:, (2 * s + 1) * P:(2 * s + 2) * P]
                return mA, mB, xA, xB

            def emit_pe_mains(s, q, close):
                mA, mB, xA, xB = mats_of(s)
                bc = st[s]["bc"]
                mm(s, q, mA, gq_ap(s, q), True, False)
                mm(s, q, mB, bc[:, q, :], False, close)

            def gq_ap8(s, q):
                b = s & 1
                return bass.AP(tensor=gsb2.tensor,
                               offset=b * GF + q * SS + PADL,
                               ap=[[2 * GF, KX], [1, W]])

            def emit_pe_cross(s, q):
                mA, mB, xA, xB = mats_of(s)
                bc = st[s]["bc"]
                mm(s, q, xA, gq_ap8(s, q + 1), False, False)
                mm(s, q, xB, bc[0:KX, q + 1, :], False, True)

            def emit_sq(s, q, qn=1):
                if "sq" not in st[s]:
                    st[s]["sq"] = work.tile([P, Q, W], F16, tag="sq",
                                            name=f"sq_{s}")
                sl = slice(q, q + qn) if qn > 1 else q
                nc.scalar.activation(st[s]["sq"][:, sl, :],
                                     psd_of(s)[:, sl, :],
                                     mybir.ActivationFunctionType.Square)

            def emit_cs(s, q):
                ivq = fmsb[:, NMAT + s * Q + q:NMAT + s * Q + q + 1]
                nc.tensor.matmul(psd_of(s)[0:1, 0, 0:W], lhsT=ivq,
                                 rhs=st[s]["sq"][:, q, :], start=(q == 0),
                                 stop=(q == Q - 1), skip_group_check=True)

            def emit_copy(s):
                nc.scalar.activation(osb[0:1, s * W:(s + 1) * W],
                                     psd_of(s)[0:1, 0, 0:W],
                                     mybir.ActivationFunctionType.Copy,
                                     scale=1.0)

            def emit_out(s):
                nc.sync.dma_start(out[s:s + 1, :], osb[0:1, s * W:(s + 1) * W])

            def emit_vload_pe():
                o0 = nc.values_load(
                    pfsb[0:1, NS * NPF + NS:NS * NPF + NS + 1].bitcast(I32),
                    engines=(mybir.EngineType.PE,),
                    skip_runtime_bounds_check=True)
                o1 = nc.values_load(
                    pfsb[0:1, NS * NPF + NS + 1:NS * NPF + NS + 2].bitcast(I32),
                    engines=(mybir.EngineType.PE,),
                    skip_runtime_bounds_check=True)
                st[NS - 1]["peoff"] = (o0, o1)

            def dyn_pe(s, q, c):
                off = st[s]["peoff"][c]
                if q:
                    off = off + (q * SS)
                dep = (s & 1) * GF + q * SS
                return bass.AP(tensor=gsb2.tensor, offset=off,
                               ap=[[2 * GF, P], [1, W]],
                               dep_tracking_offset=dep)

            def xmats_of(s):
                base = NMAT + NIV
                mains = tuple(fmsb[:, base + k * P:base + (k + 1) * P]
                              for k in range(2))
                xbase = NMAT
                crosses = tuple(xmsb[:, xbase + k * P:xbase + (k + 1) * P]
                                for k in range(2))
                return mains + crosses

            def emit_pe3_mains(s, q, close):
                mA, _, _, _ = mats_of(s)
                bc0, bc1, _, _ = xmats_of(s)
                mm(s, q, mA, gq_ap(s, q), True, False)
                mm(s, q, bc0, dyn_pe(s, q, 0), False, False)
                mm(s, q, bc1, dyn_pe(s, q, 1), False, close)

            def dyn_pe8(s, q, c):
                off = st[s]["peoff"][c]
                if q:
                    off = off + (q * SS)
                dep = (s & 1) * GF + q * SS
                return bass.AP(tensor=gsb2.tensor, offset=off,
                               ap=[[2 * GF, KX], [1, W]],
                               dep_tracking_offset=dep)

            def emit_pe3_cross(s, q):
                _, _, xA, _ = mats_of(s)
                _, _, xbc0, xbc1 = xmats_of(s)
                mm(s, q, xA, gq_ap8(s, q + 1), False, False)
                mm(s, q, xbc0, dyn_pe8(s, q + 1, 0), False, False)
                mm(s, q, xbc1, dyn_pe8(s, q + 1, 1), False, True)

            def emit_pe_all(s):
                emit_pe_mains(s, 0, False)
                emit_pe_cross(s, 0)
                emit_pe_mains(s, 1, False)
                emit_pe_cross(s, 1)
                emit_pe_mains(s, 2, False)
                emit_pe_cross(s, 2)
                emit_pe_mains(s, 3, True)

            def emit_back(s):
                emit_sq(s, 0, 2)
                emit_sq(s, 2, 2)
                for q in range(Q):
                    emit_cs(s, q)
                emit_copy(s)

            # ---- software-pipelined emission ----------------------------
            # Pool: m0 halves; DVE: comb+bc halves; ACT: squares+copies;
            # PE: delta matmuls (s3 uses dynamic-rhs, no DVE bc), colsums
            def front(s):
                emit_vload(s)
                emit_comb(s, 0, 2)
                emit_bc(s, 0, 2)
                emit_pe_mains(s, 0, False)
                emit_pe_cross(s, 0)
                emit_pe_mains(s, 1, False)
                emit_comb(s, 2, 2)
                emit_bc(s, 2, 2)
                emit_pe_cross(s, 1)
                emit_pe_mains(s, 2, False)
                emit_pe_cross(s, 2)
                emit_pe_mains(s, 3, True)

            emit_load_ch(0, 0)
            emit_consts()
            emit_load_ch(0, 1)
            emit_m0(0, 0, 2)
            emit_m0(0, 2, 2)
            emit_load_ch(1, 0)
            front(0)
            emit_load_ch(1, 1)
            emit_m0(1, 0, 2)
            emit_m0(1, 2, 2)
            emit_load_ch(2, 0)
            front(1)
            emit_back(0)
            emit_load_ch(2, 1)
            emit_m0(2, 0, 2)
            emit_m0(2, 2, 2)
            emit_load_ch(3, 0)
            front(2)
            emit_back(1)
            emit_load_ch1_part(3, 0, 2)
            emit_m0(3, 0, 2)
            emit_m0(3, 2, 1)
            emit_m0(3, 3, 1)
            emit_load_ch1_part(3, 2, 1)
            emit_load_ch1_part(3, 3, 1)
            emit_vload_pe()
            emit_comb(3, 0, 2)
            emit_pe3_mains(3, 0, False)
            emit_pe3_cross(3, 0)
            emit_pe3_mains(3, 1, False)
            emit_comb(3, 2, 1)
            emit_pe3_cross(3, 1)
            emit_pe3_mains(3, 2, False)
            emit_comb(3, 3, 1)
            emit_pe3_cross(3, 2)
            emit_pe3_mains(3, 3, True)
            emit_back(2)
            emit_sq(3, 0, 2)
            emit_sq(3, 2, 2)
            emit_cs(3, 0)
            emit_cs(3, 1)
            emit_cs(3, 2)
            emit_cs(3, 3)
            emit_copy(3)
            emit_out(0)
            emit_out(1)
            emit_out(2)
            emit_out(3)

    return nc


def _host_params(gt_sym_axis, gd_sym_axis):
    B = gt_sym_axis.shape[0]
    gt = gt_sym_axis.astype(np.float32)
    gds = gd_sym_axis.astype(np.float32)
    prm = []
    for i in range(B):
        sx = gds[i, 0]
        sy = gds[i, 1]
        dx = np.float32(-10.0) * gt[i, 0]
        dy = np.float32(10.0) * gt[i, 1]
        dy1f = np.float32(np.floor(dy))
        dx1f = np.float32(np.floor(dx))
        dy1 = int(dy1f)
        dx1 = int(dx1f)
        fy = np.float32(dy - dy1f)
        fx = np.float32(dx - dx1f)
        pos = bool(dx > 0)
        one = np.float32(1.0)
        zero = np.float32(0.0)
        if pos:
            wa = (one, zero)
            wb = (-(one - fy), -fy)
            rb, cb = dy1, dx1
            jlo, jhi = 0, W - dx1 - 1
        else:
            wa = (-fy, -(one - fy))
            wb = (one, zero)
            rb, cb = dy1 + 1, dx1
            jlo, jhi = -dx1, W
        rows = H - dy1 - 1
        cols = (W - dx1 - 1) if pos else (W + dx1)
        m = max(abs(float(sx)), abs(float(sy)), 1e-30)
        a = np.float32(float(sy) / m)
        b = np.float32(float(sx) / m)
        wf = np.array([a, b, one - fx, fx], dtype=np.float32)
        assert 0 <= rb <= PADL - 2 and -PADL + 2 <= cb <= PADR - 2
        assert 0 <= jlo <= jhi <= W
        prm.append(dict(wf=wf, wa=wa, wb=wb, rb=rb, cb=cb, jlo=jlo, jhi=jhi,
                        rows=rows, cols=cols, scale=m * m))
    return prm


def _band(pairs):
    """lhsT[k, m] = sum of w*d(k==m+o) over (o, w) pairs (within block)."""
    mat = np.zeros((P, P), np.float16)
    for o, w in pairs:
        if o >= P:
            continue
        idx = np.arange(P - o)
        mat[idx + o, idx] = np.float16(w)
    return mat


def _corner(pairs):
    """cross-seam lhsT[k, m] = w*d(k==m+o-P) for m >= P-o."""
    mat = np.zeros((P, P), np.float16)
    for o, w in pairs:
        if o <= 0:
            continue
        ms = np.arange(P - o, P)
        mat[ms + o - P, ms] = np.float16(w)
    return mat


def kernel(grid, gt_sym_axis, gd_sym_axis):
    grid = np.ascontiguousarray(grid, dtype=np.float32)
    B = grid.shape[0]
    assert grid.shape == (B, 2, H, W) and B == NS * NCORES

    if "nc" not in _CACHE:
        nc = _build_program()
        _split_multiwaits(nc)
        _CACHE["nc"] = nc
    nc = _CACHE["nc"]

    prm = _host_params(np.asarray(gt_sym_axis), np.asarray(gd_sym_axis))

    i_of_pq = np.arange(H).reshape(Q, P).T
    in_maps = []
    for c in range(NCORES):
        pfv = np.zeros((P, PFW), np.float32)
        fmv = np.zeros((P, FM), np.float16)
        xmv = np.zeros((KX, XW), np.float16)
        for s in range(NS):
            p = prm[c * NS + s]
            b = s & 1
            pfv[:, s * NPF:(s + 1) * NPF] = p["wf"][None, :]
            off = np.array([b * GF + PADL + p["cb"]], np.int32)
            pfv[0, NS * NPF + s] = off.view(np.float32)[0]
            if s == NS - 1:
                po = np.array([b * GF + PADL + p["cb"],
                               b * GF + PADL + p["cb"] + 1], np.int32)
                pfv[0, NS * NPF + NS:NS * NPF + NS + 2] = po.view(np.float32)
                wa0_, wa1_ = p["wa"]
                wb0_, wb1_ = p["wb"]
                rb_ = p["rb"]
                cb0_ = float(p["wf"][2])
                cb1_ = float(p["wf"][3])
                base = NMAT + NIV
                fmv[:, base + 0 * P:base + 1 * P] = _band(
                    [(rb_, wb0_ * cb0_), (rb_ + 1, wb1_ * cb0_)])
                fmv[:, base + 1 * P:base + 2 * P] = _band(
                    [(rb_, wb0_ * cb1_), (rb_ + 1, wb1_ * cb1_)])
                xmv[:, NMAT + 0 * P:NMAT + 1 * P] = _corner(
                    [(rb_, wb0_ * cb0_), (rb_ + 1, wb1_ * cb0_)])[:KX]
                xmv[:, NMAT + 1 * P:NMAT + 2 * P] = _corner(
                    [(rb_, wb0_ * cb1_), (rb_ + 1, wb1_ * cb1_)])[:KX]
            fmv[:, NMAT + s * Q:NMAT + (s + 1) * Q] = (
                i_of_pq < p["rows"]).astype(np.float16)
            wa0, wa1 = p["wa"]
            wb0, wb1 = p["wb"]
            rb = p["rb"]
            fmv[:, (2 * s) * P:(2 * s + 1) * P] = _band(
                [(0, wa0), (1, wa1)])
            fmv[:, (2 * s + 1) * P:(2 * s + 2) * P] = _band(
                [(rb, wb0), (rb + 1, wb1)])
            xmv[:, (2 * s) * P:(2 * s + 1) * P] = _corner(
                [(1, wa1)])[:KX]
            xmv[:, (2 * s + 1) * P:(2 * s + 2) * P] = _corner(
                [(rb, wb0), (rb + 1, wb1)])[:KX]
        in_maps.append({
            "g": grid[c * NS:(c + 1) * NS], "pfi": pfv, "fmats": fmv,
            "xm": xmv,
        })

    res = run_bass_kernel_spmd(nc, in_maps, core_ids=list(range(NCORES)))

    losses = np.zeros(B, np.float64)
    for c in range(NCORES):
        o = res.results[c]["out"]
        for s in range(NS):
            p = prm[c * NS + s]
            ssq = float(o[s, p["jlo"]:p["jhi"]].sum(dtype=np.float64))
            count = float(np.float32(p["rows"] * p["cols"]))
            losses[c * NS + s] = p["scale"] * ssq / count
    return np.float32(losses.mean())


# revision 3
# speedup vs baseline: 1.0051x; 1.0051x over previous
"""Stage-5 Trainium2 Bass kernel for nn_BilinearFullSymLoss.

Math (bc re-anchored to unshifted rows vs the stage-3 kernel):
  delta(i,j) = wA0*G(i,j) + wA1*G(i+1,j) + wB0*bc(i+rb,j) + wB1*bc(i+rb+1,j)
  bc(i,j)    = cb0*G(i, j+cb) + cb1*G(i, j+cb+1)    (col interp, same row)
  pos: wA=(1,0),         wB=(-(1-fy),-fy), rb=dy1,   cb=dx1
  neg: wA=(-fy,-(1-fy)), wB=(1,0),         rb=dy1+1, cb=dx1
       (neg evaluated at j' = j-dx1; host sums columns [-dx1, W))
  loss = m^2 * sum(valid delta^2) / (rows*cols); host does the scalar math.

Device plan per core (4 samples), no DRAM round-trip (the stage-3 kernel
wrote G to DRAM scratch and re-read dynamic windows; here every dynamic
shift is a register-offset SBUF read):
- per-sample DMA brings both channels; Pool (gpsimd) computes m0 = a*g0
  (fp16 halves); DVE scalar_tensor_tensor fuses G = b*g1 + m0 into a
  per-q-slotted fp16 tile (each 512-col block padded with 8+8 zero cols,
  so dynamic column reads never leave their slot -> race-free, and all
  out-of-range reads are finite)
- the dynamic column shift j+cb is a values_load register offset into
  that tile.  DVE computes bc = cb0*G(+cb) + cb1*G(+cb+1) via two 4x-mode
  tensor_scalar + one tensor_tensor (fp16).  Each live loaded/derived
  offset value owns a DVE register forever, so offsets are kept to two
  per sample (the walrus register allocator dies around ~24 live values)
- the row shift rb lives in the host-built banded lhsT matrices (band
  offsets {rb, rb+1}); cross-seam matrices carry the band across 128-row
  blocks and have only k < 8 nonzero rows, so they upload as [8, P]
  slices and run as k=8 matmuls; for the last block the missing cross
  rows are provably masked (neg has wB1=0)
- the LAST sample folds bc into PE entirely: delta = A@G + Bc0@G(+cb) +
  Bc1@G(+cb+1) with dynamic-offset rhs APs (values_load on the PE
  engine), so after the final DMA chunk only PE work remains ahead of
  the square/column-sum stage
- PE accumulates delta in PSUM (start/stop per bank); ACT squares
  (halves) and copies the [1,W] column sums to SBUF; PE does the
  iv-masked column sums (iv only actually masks the q=3 block); four
  per-sample output DMAs sit at the end of the SP queue so their waits
  never stall input loads.  Emission order is software-pipelined per
  engine (m0/comb/bc of sample s+1 ahead of the square/colsum backstage
  of sample s) because each sequencer is strictly in-order.
"""

import sys

sys.path.insert(0, "/opt/trn_rl_repo")

import numpy as np

import concourse.bass as bass
import concourse.tile as tile
from concourse import mybir
from concourse.bass_utils import run_bass_kernel_spmd

H = 512
W = 512
P = 128
Q = H // P
NS = 4
NCORES = 8
PADL = 8
PADR = 8
SS = PADL + W + PADR      # 528: slot size per q block
GF = Q * SS               # 2112: per-buffer free size of padded G tile
NMAT = NS * 2 * P         # mats region width in fmats
NIV = NS * Q
NXTR = 2 * P              # s3's PE-direct main matrices: Bc0, Bc1
KX = 8                    # cross matrices only have rows k < rb+1 <= 7
FM = NMAT + NIV + NXTR    # fmats cols; crosses live in the small xm tensor
XW = NMAT + 2 * P         # xm cols: xA/xB per sample + s3's xBc0/xBc1
NPF = 4                   # a, b, cb0, cb1
PFW = NS * NPF + NS + 2   # scalars + bitcast i32 DVE offsets + 2 PE offsets

F32 = mybir.dt.float32
F16 = mybir.dt.float16
I32 = mybir.dt.int32

COL_A, COL_B, COL_CB0, COL_CB1 = range(NPF)

_CACHE = {}


def _split_multiwaits(nc):
    """The staged walrus accepts one sync wait per instruction; hoist extras
    onto single-wait NoOps."""
    n = 0
    for fn in nc.m.functions:
        for bb in fn.blocks:
            newlist = []
            for ins in bb.instructions:
                si = ins.sync_info
                if si is not None and si.on_wait is not None and len(si.on_wait) > 1:
                    waits = list(si.on_wait)
                    for w in waits[:-1]:
                        n += 1
                        newlist.append(mybir.InstNoOp(
                            name=f"WSPLIT-{n}-{ins.name}", opcode="NoOp",
                            engine=ins.engine,
                            sync_info=mybir.SyncInfo(on_wait=[w], on_update=[])))
                    ins.sync_info = mybir.SyncInfo(
                        on_wait=[waits[-1]], on_update=list(si.on_update))
                newlist.append(ins)
            bb.instructions = newlist
    return n


def _build_program():
    nc = bass.Bass("TRN2", target_bir_lowering=False, debug=False)

    g = nc.dram_tensor("g", [NS, 2, H, W], F32, kind="ExternalInput")
    pfi = nc.dram_tensor("pfi", [P, PFW], F32, kind="ExternalInput")
    fmats = nc.dram_tensor("fmats", [P, FM], F16, kind="ExternalInput")
    xm = nc.dram_tensor("xm", [KX, XW], F16, kind="ExternalInput")
    out = nc.dram_tensor("out", [NS, W], F32, kind="ExternalOutput")

    with tile.TileContext(nc) as tc:
        with (
            tc.tile_pool(name="consts", bufs=1) as consts,
            tc.tile_pool(name="io", bufs=2) as io,
            tc.tile_pool(name="work", bufs=2) as work,
            tc.tile_pool(name="psd", bufs=2, space="PSUM") as psdp,
        ):
            pfsb = None
            fmsb = None
            xmsb = None
            gsb2 = None
            osb = None
            st = [dict() for _ in range(NS)]

            def pcol(s, c):
                return pfsb[:, s * NPF + c: s * NPF + c + 1]

            def emit_consts():
                nonlocal pfsb, fmsb, xmsb, gsb2, osb
                pfsb = consts.tile([P, PFW], F32)
                nc.sync.dma_start(pfsb[:], pfi[:])
                fmsb = consts.tile([P, FM], F16)
                nc.sync.dma_start(fmsb[:], fmats[:])
                xmsb = consts.tile([KX, XW], F16)
                nc.sync.dma_start(xmsb[:], xm[:])
                gsb2 = consts.tile([P, 2, GF], F16)
                nc.gpsimd.memset(gsb2[:], 0.0)
                osb = consts.tile([1, NS * W], F32)

            def emit_load_ch(s, ch):
                key = f"g{ch}t"
                if key not in st[s]:
                    st[s][key] = io.tile([P, Q, W], F32, tag=key,
                                         name=f"{key}_{s}")
                nc.sync.dma_start(
                    st[s][key][:],
                    bass.AP(tensor=g, offset=(s * 2 + ch) * H * W,
                            ap=[[W, P], [P * W, Q], [1, W]]))

            def emit_load_ch1_part(s, qlo, qn):
                key = "g1t"
                if key not in st[s]:
                    st[s][key] = io.tile([P, Q, W], F32, tag=key,
                                         name=f"{key}_{s}")
                nc.sync.dma_start(
                    st[s][key][:, qlo:qlo + qn, :],
                    bass.AP(tensor=g,
                            offset=(s * 2 + 1) * H * W + qlo * P * W,
                            ap=[[W, P], [P * W, qn], [1, W]]))

            def emit_m0(s, qlo, qn):
                if "m0" not in st[s]:
                    st[s]["m0"] = work.tile([P, Q, W], F16, tag="m0",
                                            name=f"m0_{s}")
                sl = slice(qlo, qlo + qn)
                nc.gpsimd.tensor_scalar(out=st[s]["m0"][:, sl, :],
                                        in0=st[s]["g0t"][:, sl, :],
                                        scalar1=pcol(s, COL_A), scalar2=None,
                                        op0=mybir.AluOpType.mult)

            def emit_vload(s):
                st[s]["off0"] = nc.values_load(
                    pfsb[0:1, NS * NPF + s:NS * NPF + s + 1].bitcast(I32),
                    engines=(mybir.EngineType.DVE,),
                    skip_runtime_bounds_check=True)

            def emit_comb(s, qlo, qn, eng=None):
                b = s & 1
                sl = slice(qlo, qlo + qn) if qn > 1 else qlo
                gap = ([[2 * GF, P], [SS, qn], [1, W]] if qn > 1
                       else [[2 * GF, P], [1, W]])
                gout = bass.AP(tensor=gsb2.tensor,
                               offset=b * GF + qlo * SS + PADL, ap=gap)
                (eng or nc.vector).scalar_tensor_tensor(
                    out=gout, in0=st[s]["g1t"][:, sl, :],
                    scalar=pcol(s, COL_B), in1=st[s]["m0"][:, sl, :],
                    op0=mybir.AluOpType.mult, op1=mybir.AluOpType.add)

            def emit_bc(s, qlo, qn):
                b = s & 1
                for nm in ("t5", "t6", "bc"):
                    if nm not in st[s]:
                        st[s][nm] = work.tile([P, Q, W], F16, tag=nm,
                                              name=f"{nm}_{s}")
                t5, t6, bc = st[s]["t5"], st[s]["t6"], st[s]["bc"]
                sl = slice(qlo, qlo + qn) if qn > 1 else qlo
                dep = b * GF + qlo * SS
                base = (st[s]["off0"] + (qlo * SS)) if qlo else st[s]["off0"]

                def dyn(off):
                    ap = ([[2 * GF, P], [SS, qn], [1, W]] if qn > 1
                          else [[2 * GF, P], [1, W]])
                    return bass.AP(tensor=gsb2.tensor, offset=off,
                                   ap=ap, dep_tracking_offset=dep)

                nc.vector.tensor_scalar(
                    out=t5[:, sl, :], in0=dyn(base + 1),
                    scalar1=pcol(s, COL_CB1), scalar2=None,
                    op0=mybir.AluOpType.mult)
                nc.vector.tensor_scalar(
                    out=t6[:, sl, :], in0=dyn(base),
                    scalar1=pcol(s, COL_CB0), scalar2=None,
                    op0=mybir.AluOpType.mult)
                nc.vector.tensor_tensor(
                    out=bc[:, sl, :], in0=t5[:, sl, :], in1=t6[:, sl, :],
                    op=mybir.AluOpType.add)

            def gq_ap(s, q):
                b = s & 1
                return bass.AP(tensor=gsb2.tensor,
                               offset=b * GF + q * SS + PADL,
                               ap=[[2 * GF, P], [1, W]])

            def psd_of(s):
                if "psd" not in st[s]:
                    st[s]["psd"] = psdp.tile([P, Q, W], F32, tag="psd",
                                             name=f"psd_{s}")
                return st[s]["psd"]

            def mm(s, q, lhsT, rhs, st_, sp):
                nc.tensor.matmul(psd_of(s)[:, q, :], lhsT=lhsT, rhs=rhs,
                                 start=st_, stop=sp, skip_group_check=True)

            def mats_of(s):
                mA = fmsb[:, (2 * s) * P:(2 * s + 1) * P]
                mB = fmsb[:, (2 * s + 1) * P:(2 * s + 2) * P]
                xA = xmsb[:, (2 * s) * P:(2 * s + 1) * P]
                xB = xmsb[:, (2 * s + 1) * P:(2 * s + 2) * P]
                return mA, mB, xA, xB

            def emit_pe_mains(s, q, close):
                mA, mB, xA, xB = mats_of(s)
                bc = st[s]["bc"]
                mm(s, q, mA, gq_ap(s, q), True, False)
                mm(s, q, mB, bc[:, q, :], False, close)

            def gq_ap8(s, q):
                b = s & 1
                return bass.AP(tensor=gsb2.tensor,
                               offset=b * GF + q * SS + PADL,
                               ap=[[2 * GF, KX], [1, W]])

            def emit_pe_cross(s, q):
                mA, mB, xA, xB = mats_of(s)
                bc = st[s]["bc"]
                mm(s, q, xA, gq_ap8(s, q + 1), False, False)
                mm(s, q, xB, bc[0:KX, q + 1, :], False, True)

            def emit_sq(s, q, qn=1):
                if "sq" not in st[s]:
                    st[s]["sq"] = work.tile([P, Q, W], F16, tag="sq",
                                            name=f"sq_{s}")
                sl = slice(q, q + qn) if qn > 1 else q
                nc.scalar.activation(st[s]["sq"][:, sl, :],
                                     psd_of(s)[:, sl, :],
                                     mybir.ActivationFunctionType.Square)

            def emit_cs(s, q):
                ivq = fmsb[:, NMAT + s * Q + q:NMAT + s * Q + q + 1]
                nc.tensor.matmul(psd_of(s)[0:1, 0, 0:W], lhsT=ivq,
                                 rhs=st[s]["sq"][:, q, :], start=(q == 0),
                                 stop=(q == Q - 1), skip_group_check=True)

            def emit_copy(s):
                nc.scalar.activation(osb[0:1, s * W:(s + 1) * W],
                                     psd_of(s)[0:1, 0, 0:W],
                                     mybir.ActivationFunctionType.Copy,
                                     scale=1.0)

            def emit_out(s):
                nc.sync.dma_start(out[s:s + 1, :], osb[0:1, s * W:(s + 1) * W])

            def emit_vload_pe():
                o0 = nc.values_load(
                    pfsb[0:1, NS * NPF + NS:NS * NPF + NS + 1].bitcast(I32),
                    engines=(mybir.EngineType.PE,),
                    skip_runtime_bounds_check=True)
                o1 = nc.values_load(
                    pfsb[0:1, NS * NPF + NS + 1:NS * NPF + NS + 2].bitcast(I32),
                    engines=(mybir.EngineType.PE,),
                    skip_runtime_bounds_check=True)
                st[NS - 1]["peoff"] = (o0, o1)

            def dyn_pe(s, q, c):
                off = st[s]["peoff"][c]
                if q:
                    off = off + (q * SS)
                dep = (s & 1) * GF + q * SS
                return bass.AP(tensor=gsb2.tensor, offset=off,
                               ap=[[2 * GF, P], [1, W]],
                               dep_tracking_offset=dep)

            def xmats_of(s):
                base = NMAT + NIV
                mains = tuple(fmsb[:, base + k * P:base + (k + 1) * P]
                              for k in range(2))
                xbase = NMAT
                crosses = tuple(xmsb[:, xbase + k * P:xbase + (k + 1) * P]
                                for k in range(2))
                return mains + crosses

            def emit_pe3_mains(s, q, close):
                mA, _, _, _ = mats_of(s)
                bc0, bc1, _, _ = xmats_of(s)
                mm(s, q, mA, gq_ap(s, q), True, False)
                mm(s, q, bc0, dyn_pe(s, q, 0), False, False)
                mm(s, q, bc1, dyn_pe(s, q, 1), False, close)

            def dyn_pe8(s, q, c):
                off = st[s]["peoff"][c]
                if q:
                    off = off + (q * SS)
                dep = (s & 1) * GF + q * SS
                return bass.AP(tensor=gsb2.tensor, offset=off,
                               ap=[[2 * GF, KX], [1, W]],
                               dep_tracking_offset=dep)

            def emit_pe3_cross(s, q):
                _, _, xA, _ = mats_of(s)
                _, _, xbc0, xbc1 = xmats_of(s)
                mm(s, q, xA, gq_ap8(s, q + 1), False, False)
                mm(s, q, xbc0, dyn_pe8(s, q + 1, 0), False, False)
                mm(s, q, xbc1, dyn_pe8(s, q + 1, 1), False, True)

            def emit_pe_all(s):
                emit_pe_mains(s, 0, False)
                emit_pe_cross(s, 0)
                emit_pe_mains(s, 1, False)
                emit_pe_cross(s, 1)
                emit_pe_mains(s, 2, False)
                emit_pe_cross(s, 2)
                emit_pe_mains(s, 3, True)

            def emit_back(s):
                emit_sq(s, 0, 2)
                emit_sq(s, 2, 2)
                for q in range(Q):
                    emit_cs(s, q)
                emit_copy(s)

            # ---- software-pipelined emission ----------------------------
            # Pool: m0 halves; DVE: comb+bc halves; ACT: squares+copies;
            # PE: delta matmuls (s3 uses dynamic-rhs, no DVE bc), colsums
            def front(s):
                emit_vload(s)
                emit_comb(s, 0, 2)
                emit_bc(s, 0, 2)
                emit_pe_mains(s, 0, False)
                emit_pe_cross(s, 0)
                emit_pe_mains(s, 1, False)
                emit_comb(s, 2, 2)
                emit_bc(s, 2, 2)
                emit_pe_cross(s, 1)
                emit_pe_mains(s, 2, False)
                emit_pe_cross(s, 2)
                emit_pe_mains(s, 3, True)

            emit_load_ch(0, 0)
            emit_consts()
            emit_load_ch(0, 1)
            emit_m0(0, 0, 2)
            emit_m0(0, 2, 2)
            emit_load_ch(1, 0)
            front(0)
            emit_load_ch(1, 1)
            emit_m0(1, 0, 2)
            emit_m0(1, 2, 2)
            emit_load_ch(2, 0)
            front(1)
            emit_back(0)
            emit_load_ch(2, 1)
            emit_m0(2, 0, 2)
            emit_m0(2, 2, 2)
            emit_load_ch(3, 0)
            front(2)
            emit_back(1)
            emit_load_ch1_part(3, 0, 2)
            emit_m0(3, 0, 2)
            emit_m0(3, 2, 1)
            emit_m0(3, 3, 1)
            emit_load_ch1_part(3, 2, 1)
            emit_load_ch1_part(3, 3, 1)
            emit_vload_pe()
            emit_comb(3, 0, 2)
            emit_pe3_mains(3, 0, False)
            emit_pe3_cross(3, 0)
            emit_pe3_mains(3, 1, False)
            emit_comb(3, 2, 1)
            emit_pe3_cross(3, 1)
            emit_pe3_mains(3, 2, False)
            emit_comb(3, 3, 1)
            emit_pe3_cross(3, 2)
            emit_pe3_mains(3, 3, True)
            emit_back(2)
            emit_sq(3, 0, 2)
            emit_sq(3, 2, 2)
            emit_cs(3, 0)
            emit_cs(3, 1)
            emit_cs(3, 2)
            emit_cs(3, 3)
            emit_copy(3)
            emit_out(0)
            emit_out(1)
            emit_out(2)
            emit_out(3)

    return nc


def _host_params(gt_sym_axis, gd_sym_axis):
    B = gt_sym_axis.shape[0]
    gt = gt_sym_axis.astype(np.float32)
    gds = gd_sym_axis.astype(np.float32)
    prm = []
    for i in range(B):
        sx = gds[i, 0]
        sy = gds[i, 1]
        dx = np.float32(-10.0) * gt[i, 0]
        dy = np.float32(10.0) * gt[i, 1]
        dy1f = np.float32(np.floor(dy))
        dx1f = np.float32(np.floor(dx))
        dy1 = int(dy1f)
        dx1 = int(dx1f)
        fy = np.float32(dy - dy1f)
        fx = np.float32(dx - dx1f)
        pos = bool(dx > 0)
        one = np.float32(1.0)
        zero = np.float32(0.0)
        if pos:
            wa = (one, zero)
            wb = (-(one - fy), -fy)
            rb, cb = dy1, dx1
            jlo, jhi = 0, W - dx1 - 1
        else:
            wa = (-fy, -(one - fy))
            wb = (one, zero)
            rb, cb = dy1 + 1, dx1
            jlo, jhi = -dx1, W
        rows = H - dy1 - 1
        cols = (W - dx1 - 1) if pos else (W + dx1)
        m = max(abs(float(sx)), abs(float(sy)), 1e-30)
        a = np.float32(float(sy) / m)
        b = np.float32(float(sx) / m)
        wf = np.array([a, b, one - fx, fx], dtype=np.float32)
        assert 0 <= rb <= PADL - 2 and -PADL + 2 <= cb <= PADR - 2
        assert 0 <= jlo <= jhi <= W
        prm.append(dict(wf=wf, wa=wa, wb=wb, rb=rb, cb=cb, jlo=jlo, jhi=jhi,
                        rows=rows, cols=cols, scale=m * m))
    return prm


def _band(pairs):
    """lhsT[k, m] = sum of w*d(k==m+o) over (o, w) pairs (within block)."""
    mat = np.zeros((P, P), np.float16)
    for o, w in pairs:
        if o >= P:
            continue
        idx = np.arange(P - o)
        mat[idx + o, idx] = np.float16(w)
    return mat


def _corner(pairs):
    """cross-seam lhsT[k, m] = w*d(k==m+o-P) for m >= P-o."""
    mat = np.zeros((P, P), np.float16)
    for o, w in pairs:
        if o <= 0:
            continue
        ms = np.arange(P - o, P)
        mat[ms + o - P, ms] = np.float16(w)
    return mat


def kernel(grid, gt_sym_axis, gd_sym_axis):
    grid = np.ascontiguousarray(grid, dtype=np.float32)
    B = grid.shape[0]
    assert grid.shape == (B, 2, H, W) and B == NS * NCORES

    if "nc" not in _CACHE:
        nc = _build_program()
        _split_multiwaits(nc)
        _CACHE["nc"] = nc
    nc = _CACHE["nc"]

    prm = _host_params(np.asarray(gt_sym_axis), np.asarray(gd_sym_axis))

    i_of_pq = np.arange(H).reshape(Q, P).T
    in_maps = []
    for c in range(NCORES):
        pfv = np.zeros((P, PFW), np.float32)
        fmv = np.zeros((P, FM), np.float16)
        xmv = np.zeros((KX, XW), np.float16)
        for s in range(NS):
            p = prm[c * NS + s]
            b = s & 1
            pfv[:, s * NPF:(s + 1) * NPF] = p["wf"][None, :]
            off = np.array([b * GF + PADL + p["cb"]], np.int32)
            pfv[0, NS * NPF + s] = off.view(np.float32)[0]
            if s == NS - 1:
                po = np.array([b * GF + PADL + p["cb"],
                               b * GF + PADL + p["cb"] + 1], np.int32)
                pfv[0, NS * NPF + NS:NS * NPF + NS + 2] = po.view(np.float32)
                wa0_, wa1_ = p["wa"]
                wb0_, wb1_ = p["wb"]
                rb_ = p["rb"]
                cb0_ = float(p["wf"][2])
                cb1_ = float(p["wf"][3])
                base = NMAT + NIV
                fmv[:, base + 0 * P:base + 1 * P] = _band(
                    [(rb_, wb0_ * cb0_), (rb_ + 1, wb1_ * cb0_)])
                fmv[:, base + 1 * P:base + 2 * P] = _band(
                    [(rb_, wb0_ * cb1_), (rb_ + 1, wb1_ * cb1_)])
                xmv[:, NMAT + 0 * P:NMAT + 1 * P] = _corner(
                    [(rb_, wb0_ * cb0_), (rb_ + 1, wb1_ * cb0_)])[:KX]
                xmv[:, NMAT + 1 * P:NMAT + 2 * P] = _corner(
                    [(rb_, wb0_ * cb1_), (rb_ + 1, wb1_ * cb1_)])[:KX]
            fmv[:, NMAT + s * Q:NMAT + (s + 1) * Q] = (
                i_of_pq < p["rows"]).astype(np.float16)
            wa0, wa1 = p["wa"]
            wb0, wb1 = p["wb"]
            rb = p["rb"]
            fmv[:, (2 * s) * P:(2 * s + 1) * P] = _band(
                [(0, wa0), (1, wa1)])
            fmv[:, (2 * s + 1) * P:(2 * s + 2) * P] = _band(
                [(rb, wb0), (rb + 1, wb1)])
            xmv[:, (2 * s) * P:(2 * s + 1) * P] = _corner(
                [(1, wa1)])[:KX]
            xmv[:, (2 * s + 1) * P:(2 * s + 2) * P] = _corner(
                [(rb, wb0), (rb + 1, wb1)])[:KX]
        in_maps.append({
            "g": grid[c * NS:(c + 1) * NS], "pfi": pfv, "fmats": fmv,
            "xm": xmv,
        })

    res = run_bass_kernel_spmd(nc, in_maps, core_ids=list(range(NCORES)))

    losses = np.zeros(B, np.float64)
    for c in range(NCORES):
        o = res.results[c]["out"]
        for s in range(NS):
            p = prm[c * NS + s]
            ssq = float(o[s, p["jlo"]:p["jhi"]].sum(dtype=np.float64))
            count = float(np.float32(p["rows"] * p["cols"]))
            losses[c * NS + s] = p["scale"] * ssq / count
    return np.float32(losses.mean())


# revision 4
# speedup vs baseline: 1.1885x; 1.1825x over previous
"""Stage-5 Trainium2 Bass kernel for nn_BilinearFullSymLoss.

Math (bc re-anchored to unshifted rows vs the stage-3 kernel):
  delta(i,j) = wA0*G(i,j) + wA1*G(i+1,j) + wB0*bc(i+rb,j) + wB1*bc(i+rb+1,j)
  bc(i,j)    = cb0*G(i, j+cb) + cb1*G(i, j+cb+1)    (col interp, same row)
  pos: wA=(1,0),         wB=(-(1-fy),-fy), rb=dy1,   cb=dx1
  neg: wA=(-fy,-(1-fy)), wB=(1,0),         rb=dy1+1, cb=dx1
       (neg evaluated at j' = j-dx1; host sums columns [-dx1, W))
  loss = m^2 * sum(valid delta^2) / (rows*cols); host does the scalar math.

Device plan per core (4 samples), no DRAM round-trip (the stage-3 kernel
wrote G to DRAM scratch and re-read dynamic windows; here every dynamic
shift is a register-offset SBUF read):
- per-sample DMA brings both channels; Pool (gpsimd) computes m0 = a*g0
  (fp16 halves); DVE scalar_tensor_tensor fuses G = b*g1 + m0 into a
  per-q-slotted fp16 tile (each 512-col block padded with 8+8 zero cols,
  so dynamic column reads never leave their slot -> race-free, and all
  out-of-range reads are finite)
- the dynamic column shift j+cb is a values_load register offset into
  that tile.  DVE computes bc = cb0*G(+cb) + cb1*G(+cb+1) via two 4x-mode
  tensor_scalar + one tensor_tensor (fp16).  Each live loaded/derived
  offset value owns a DVE register forever, so offsets are kept to two
  per sample (the walrus register allocator dies around ~24 live values)
- the row shift rb lives in the host-built banded lhsT matrices (band
  offsets {rb, rb+1}); cross-seam matrices carry the band across 128-row
  blocks and have only k < 8 nonzero rows, so they upload as [8, P]
  slices and run as k=8 matmuls; for the last block the missing cross
  rows are provably masked (neg has wB1=0)
- the LAST sample folds bc into PE entirely: delta = A@G + Bc0@G(+cb) +
  Bc1@G(+cb+1) with dynamic-offset rhs APs (values_load on the PE
  engine), so after the final DMA chunk only PE work remains ahead of
  the square/column-sum stage
- PE accumulates delta in PSUM (start/stop per bank); ACT squares
  (halves) and copies the [1,W] column sums to SBUF; PE does the
  iv-masked column sums (iv only actually masks the q=3 block); four
  per-sample output DMAs sit at the end of the SP queue so their waits
  never stall input loads.  Emission order is software-pipelined per
  engine (m0/comb/bc of sample s+1 ahead of the square/colsum backstage
  of sample s) because each sequencer is strictly in-order.
"""

import sys

sys.path.insert(0, "/opt/trn_rl_repo")

import numpy as np

import concourse.bass as bass
import concourse.tile as tile
from concourse import mybir
from concourse.bass_utils import run_bass_kernel_spmd

H = 512
W = 512
P = 128
Q = H // P
NS = 4
NCORES = 8
PADL = 8
PADR = 8
SS = PADL + W + PADR      # 528: slot size per q block
GF = Q * SS               # 2112: per-buffer free size of padded G tile
NMAT = NS * 2 * P         # mats region width in fmats
NIV = NS * Q
NXTR = 2 * P              # s3's PE-direct main matrices: Bc0, Bc1
KX = 8                    # cross matrices only have rows k < rb+1 <= 7
FM = NMAT + NIV + NXTR    # fmats cols; crosses live in the small xm tensor
XW = NMAT + 2 * P         # xm cols: xA/xB per sample + s3's xBc0/xBc1
NPF = 4                   # a, b, cb0, cb1
PFW = NS * NPF + NS + 2   # scalars + bitcast i32 DVE offsets + 2 PE offsets

F32 = mybir.dt.float32
F16 = mybir.dt.float16
I32 = mybir.dt.int32

COL_A, COL_B, COL_CB0, COL_CB1 = range(NPF)

_CACHE = {}


def _split_multiwaits(nc):
    """The staged walrus accepts one sync wait per instruction; hoist extras
    onto single-wait NoOps."""
    n = 0
    for fn in nc.m.functions:
        for bb in fn.blocks:
            newlist = []
            for ins in bb.instructions:
                si = ins.sync_info
                if si is not None and si.on_wait is not None and len(si.on_wait) > 1:
                    waits = list(si.on_wait)
                    for w in waits[:-1]:
                        n += 1
                        newlist.append(mybir.InstNoOp(
                            name=f"WSPLIT-{n}-{ins.name}", opcode="NoOp",
                            engine=ins.engine,
                            sync_info=mybir.SyncInfo(on_wait=[w], on_update=[])))
                    ins.sync_info = mybir.SyncInfo(
                        on_wait=[waits[-1]], on_update=list(si.on_update))
                newlist.append(ins)
            bb.instructions = newlist
    return n


def _build_program():
    nc = bass.Bass("TRN2", target_bir_lowering=False, debug=False)

    g = nc.dram_tensor("g", [NS, 2, H, W], F32, kind="ExternalInput")
    pfi = nc.dram_tensor("pfi", [P, PFW], F32, kind="ExternalInput")
    fmats = nc.dram_tensor("fmats", [P, FM], F16, kind="ExternalInput")
    xm = nc.dram_tensor("xm", [KX, XW], F16, kind="ExternalInput")
    out = nc.dram_tensor("out", [NS, W], F32, kind="ExternalOutput")

    with tile.TileContext(nc) as tc:
        with (
            tc.tile_pool(name="consts", bufs=1) as consts,
            tc.tile_pool(name="io", bufs=2) as io,
            tc.tile_pool(name="work", bufs=2) as work,
            tc.tile_pool(name="psd", bufs=2, space="PSUM") as psdp,
        ):
            pfsb = None
            fmsb = None
            xmsb = None
            gsb2 = None
            osb = None
            st = [dict() for _ in range(NS)]

            def pcol(s, c):
                return pfsb[:, s * NPF + c: s * NPF + c + 1]

            def emit_consts():
                nonlocal pfsb, fmsb, xmsb, gsb2, osb
                pfsb = consts.tile([P, PFW], F32)
                nc.sync.dma_start(pfsb[:], pfi[:])
                fmsb = consts.tile([P, FM], F16)
                nc.sync.dma_start(fmsb[:], fmats[:])
                xmsb = consts.tile([KX, XW], F16)
                nc.sync.dma_start(xmsb[:], xm[:])
                gsb2 = consts.tile([P, 2, GF], F16)
                nc.gpsimd.memset(gsb2[:], 0.0)
                osb = consts.tile([1, NS * W], F32)

            def emit_load_ch(s, ch):
                key = f"g{ch}t"
                if key not in st[s]:
                    st[s][key] = io.tile([P, Q, W], F32, tag=key,
                                         name=f"{key}_{s}")
                nc.sync.dma_start(
                    st[s][key][:],
                    bass.AP(tensor=g, offset=(s * 2 + ch) * H * W,
                            ap=[[W, P], [P * W, Q], [1, W]]))

            def emit_load_ch1_part(s, qlo, qn):
                key = "g1t"
                if key not in st[s]:
                    st[s][key] = io.tile([P, Q, W], F32, tag=key,
                                         name=f"{key}_{s}")
                nc.sync.dma_start(
                    st[s][key][:, qlo:qlo + qn, :],
                    bass.AP(tensor=g,
                            offset=(s * 2 + 1) * H * W + qlo * P * W,
                            ap=[[W, P], [P * W, qn], [1, W]]))

            def emit_m0(s, qlo, qn):
                if "m0" not in st[s]:
                    st[s]["m0"] = work.tile([P, Q, W], F16, tag="m0",
                                            name=f"m0_{s}")
                sl = slice(qlo, qlo + qn)
                nc.gpsimd.tensor_scalar(out=st[s]["m0"][:, sl, :],
                                        in0=st[s]["g0t"][:, sl, :],
                                        scalar1=pcol(s, COL_A), scalar2=None,
                                        op0=mybir.AluOpType.mult)

            def emit_vload(s):
                st[s]["off0"] = nc.values_load(
                    pfsb[0:1, NS * NPF + s:NS * NPF + s + 1].bitcast(I32),
                    engines=(mybir.EngineType.DVE,),
                    skip_runtime_bounds_check=True)

            def emit_comb(s, qlo, qn, eng=None):
                b = s & 1
                sl = slice(qlo, qlo + qn) if qn > 1 else qlo
                gap = ([[2 * GF, P], [SS, qn], [1, W]] if qn > 1
                       else [[2 * GF, P], [1, W]])
                gout = bass.AP(tensor=gsb2.tensor,
                               offset=b * GF + qlo * SS + PADL, ap=gap)
                (eng or nc.vector).scalar_tensor_tensor(
                    out=gout, in0=st[s]["g1t"][:, sl, :],
                    scalar=pcol(s, COL_B), in1=st[s]["m0"][:, sl, :],
                    op0=mybir.AluOpType.mult, op1=mybir.AluOpType.add)

            def emit_bc(s, qlo, qn):
                b = s & 1
                for nm in ("t5", "t6", "bc"):
                    if nm not in st[s]:
                        st[s][nm] = work.tile([P, Q, W], F16, tag=nm,
                                              name=f"{nm}_{s}")
                t5, t6, bc = st[s]["t5"], st[s]["t6"], st[s]["bc"]
                sl = slice(qlo, qlo + qn) if qn > 1 else qlo
                dep = b * GF + qlo * SS
                base = (st[s]["off0"] + (qlo * SS)) if qlo else st[s]["off0"]

                def dyn(off):
                    ap = ([[2 * GF, P], [SS, qn], [1, W]] if qn > 1
                          else [[2 * GF, P], [1, W]])
                    return bass.AP(tensor=gsb2.tensor, offset=off,
                                   ap=ap, dep_tracking_offset=dep)

                nc.vector.tensor_scalar(
                    out=t5[:, sl, :], in0=dyn(base + 1),
                    scalar1=pcol(s, COL_CB1), scalar2=None,
                    op0=mybir.AluOpType.mult)
                nc.vector.tensor_scalar(
                    out=t6[:, sl, :], in0=dyn(base),
                    scalar1=pcol(s, COL_CB0), scalar2=None,
                    op0=mybir.AluOpType.mult)
                nc.vector.tensor_tensor(
                    out=bc[:, sl, :], in0=t5[:, sl, :], in1=t6[:, sl, :],
                    op=mybir.AluOpType.add)

            def gq_ap(s, q):
                b = s & 1
                return bass.AP(tensor=gsb2.tensor,
                               offset=b * GF + q * SS + PADL,
                               ap=[[2 * GF, P], [1, W]])

            def psd_of(s):
                if "psd" not in st[s]:
                    st[s]["psd"] = psdp.tile([P, Q, W], F32, tag="psd",
                                             name=f"psd_{s}")
                return st[s]["psd"]

            def mm(s, q, lhsT, rhs, st_, sp):
                nc.tensor.matmul(psd_of(s)[:, q, :], lhsT=lhsT, rhs=rhs,
                                 start=st_, stop=sp, skip_group_check=True)

            def mats_of(s):
                mA = fmsb[:, (2 * s) * P:(2 * s + 1) * P]
                mB = fmsb[:, (2 * s + 1) * P:(2 * s + 2) * P]
                xA = xmsb[:, (2 * s) * P:(2 * s + 1) * P]
                xB = xmsb[:, (2 * s + 1) * P:(2 * s + 2) * P]
                return mA, mB, xA, xB

            def emit_pe_mains(s, q, close):
                mA, mB, xA, xB = mats_of(s)
                bc = st[s]["bc"]
                mm(s, q, mA, gq_ap(s, q), True, False)
                mm(s, q, mB, bc[:, q, :], False, close)

            def gq_ap8(s, q):
                b = s & 1
                return bass.AP(tensor=gsb2.tensor,
                               offset=b * GF + q * SS + PADL,
                               ap=[[2 * GF, KX], [1, W]])

            def emit_pe_cross(s, q):
                mA, mB, xA, xB = mats_of(s)
                bc = st[s]["bc"]
                mm(s, q, xA, gq_ap8(s, q + 1), False, False)
                mm(s, q, xB, bc[0:KX, q + 1, :], False, True)

            def emit_sq(s, q, qn=1):
                # separate tiles per half: subtile deps are imprecise, so
                # one tile would make the first colsums wait the last square
                key = "sq0" if q < 2 else "sq1"
                if key not in st[s]:
                    st[s][key] = work.tile([P, 2, W], F16, tag=key,
                                           name=f"{key}_{s}")
                sl = slice(q % 2, q % 2 + qn) if qn > 1 else q % 2
                nc.scalar.activation(st[s][key][:, sl, :],
                                     psd_of(s)[:, slice(q, q + qn) if qn > 1
                                               else q, :],
                                     mybir.ActivationFunctionType.Square)

            def emit_cs(s, q):
                ivq = fmsb[:, NMAT + s * Q + q:NMAT + s * Q + q + 1]
                sq = st[s]["sq0" if q < 2 else "sq1"]
                nc.tensor.matmul(psd_of(s)[0:1, 0, 0:W], lhsT=ivq,
                                 rhs=sq[:, q % 2, :], start=(q == 0),
                                 stop=(q == Q - 1), skip_group_check=True)

            def emit_copy(s):
                # DVE: idle in the tail, and a copy on ACT wedges between
                # the last sample's squares on the in-order ACT queue
                nc.vector.tensor_copy(osb[0:1, s * W:(s + 1) * W],
                                      psd_of(s)[0:1, 0, 0:W])

            def emit_out(s):
                nc.sync.dma_start(out[s:s + 1, :], osb[0:1, s * W:(s + 1) * W])

            def emit_vload_pe():
                o0 = nc.values_load(
                    pfsb[0:1, NS * NPF + NS:NS * NPF + NS + 1].bitcast(I32),
                    engines=(mybir.EngineType.PE,),
                    skip_runtime_bounds_check=True)
                o1 = nc.values_load(
                    pfsb[0:1, NS * NPF + NS + 1:NS * NPF + NS + 2].bitcast(I32),
                    engines=(mybir.EngineType.PE,),
                    skip_runtime_bounds_check=True)
                st[NS - 1]["peoff"] = (o0, o1)

            def dyn_pe(s, q, c):
                off = st[s]["peoff"][c]
                if q:
                    off = off + (q * SS)
                dep = (s & 1) * GF + q * SS
                return bass.AP(tensor=gsb2.tensor, offset=off,
                               ap=[[2 * GF, P], [1, W]],
                               dep_tracking_offset=dep)

            def xmats_of(s):
                base = NMAT + NIV
                mains = tuple(fmsb[:, base + k * P:base + (k + 1) * P]
                              for k in range(2))
                xbase = NMAT
                crosses = tuple(xmsb[:, xbase + k * P:xbase + (k + 1) * P]
                                for k in range(2))
                return mains + crosses

            def emit_pe3_mains(s, q, close):
                mA, _, _, _ = mats_of(s)
                bc0, bc1, _, _ = xmats_of(s)
                mm(s, q, mA, gq_ap(s, q), True, False)
                mm(s, q, bc0, dyn_pe(s, q, 0), False, False)
                mm(s, q, bc1, dyn_pe(s, q, 1), False, close)

            def dyn_pe8(s, q, c):
                off = st[s]["peoff"][c]
                if q:
                    off = off + (q * SS)
                dep = (s & 1) * GF + q * SS
                return bass.AP(tensor=gsb2.tensor, offset=off,
                               ap=[[2 * GF, KX], [1, W]],
                               dep_tracking_offset=dep)

            def emit_pe3_cross(s, q):
                _, _, xA, _ = mats_of(s)
                _, _, xbc0, xbc1 = xmats_of(s)
                mm(s, q, xA, gq_ap8(s, q + 1), False, False)
                mm(s, q, xbc0, dyn_pe8(s, q + 1, 0), False, False)
                mm(s, q, xbc1, dyn_pe8(s, q + 1, 1), False, True)

            def emit_pe_all(s):
                emit_pe_mains(s, 0, False)
                emit_pe_cross(s, 0)
                emit_pe_mains(s, 1, False)
                emit_pe_cross(s, 1)
                emit_pe_mains(s, 2, False)
                emit_pe_cross(s, 2)
                emit_pe_mains(s, 3, True)

            def emit_back(s):
                emit_sq(s, 0, 2)
                emit_sq(s, 2, 2)
                for q in range(Q):
                    emit_cs(s, q)
                emit_copy(s)

            # ---- software-pipelined emission ----------------------------
            # Pool: m0 halves; DVE: comb+bc halves; ACT: squares+copies;
            # PE: delta matmuls (s3 uses dynamic-rhs, no DVE bc), colsums
            def front(s):
                emit_vload(s)
                emit_comb(s, 0, 2)
                emit_bc(s, 0, 2)
                emit_pe_mains(s, 0, False)
                emit_pe_cross(s, 0)
                emit_pe_mains(s, 1, False)
                emit_comb(s, 2, 2)
                emit_bc(s, 2, 2)
                emit_pe_cross(s, 1)
                emit_pe_mains(s, 2, False)
                emit_pe_cross(s, 2)
                emit_pe_mains(s, 3, True)

            emit_load_ch(0, 0)
            emit_consts()
            emit_load_ch(0, 1)
            emit_m0(0, 0, 2)
            emit_m0(0, 2, 2)
            emit_load_ch(1, 0)
            front(0)
            emit_load_ch(1, 1)
            emit_m0(1, 0, 2)
            emit_m0(1, 2, 2)
            emit_load_ch(2, 0)
            front(1)
            emit_back(0)
            emit_load_ch(2, 1)
            emit_m0(2, 0, 2)
            emit_m0(2, 2, 2)
            emit_load_ch(3, 0)
            front(2)
            emit_back(1)
            emit_load_ch1_part(3, 0, 2)
            emit_m0(3, 0, 2)
            emit_m0(3, 2, 1)
            emit_m0(3, 3, 1)
            emit_load_ch1_part(3, 2, 1)
            emit_load_ch1_part(3, 3, 1)
            emit_vload_pe()
            emit_comb(3, 0, 2)
            emit_pe3_mains(3, 0, False)
            emit_pe3_cross(3, 0)
            emit_pe3_mains(3, 1, False)
            emit_comb(3, 2, 1)
            emit_pe3_cross(3, 1)
            emit_pe3_mains(3, 2, False)
            emit_comb(3, 3, 1)
            emit_pe3_cross(3, 2)
            emit_pe3_mains(3, 3, True)
            emit_back(2)
            emit_sq(3, 0, 2)
            emit_sq(3, 2, 2)
            emit_cs(3, 0)
            emit_cs(3, 1)
            emit_cs(3, 2)
            emit_cs(3, 3)
            emit_copy(3)
            emit_out(0)
            emit_out(1)
            emit_out(2)
            emit_out(3)

    return nc


def _host_params(gt_sym_axis, gd_sym_axis):
    B = gt_sym_axis.shape[0]
    gt = gt_sym_axis.astype(np.float32)
    gds = gd_sym_axis.astype(np.float32)
    prm = []
    for i in range(B):
        sx = gds[i, 0]
        sy = gds[i, 1]
        dx = np.float32(-10.0) * gt[i, 0]
        dy = np.float32(10.0) * gt[i, 1]
        dy1f = np.float32(np.floor(dy))
        dx1f = np.float32(np.floor(dx))
        dy1 = int(dy1f)
        dx1 = int(dx1f)
        fy = np.float32(dy - dy1f)
        fx = np.float32(dx - dx1f)
        pos = bool(dx > 0)
        one = np.float32(1.0)
        zero = np.float32(0.0)
        if pos:
            wa = (one, zero)
            wb = (-(one - fy), -fy)
            rb, cb = dy1, dx1
            jlo, jhi = 0, W - dx1 - 1
        else:
            wa = (-fy, -(one - fy))
            wb = (one, zero)
            rb, cb = dy1 + 1, dx1
            jlo, jhi = -dx1, W
        rows = H - dy1 - 1
        cols = (W - dx1 - 1) if pos else (W + dx1)
        m = max(abs(float(sx)), abs(float(sy)), 1e-30)
        a = np.float32(float(sy) / m)
        b = np.float32(float(sx) / m)
        wf = np.array([a, b, one - fx, fx], dtype=np.float32)
        assert 0 <= rb <= PADL - 2 and -PADL + 2 <= cb <= PADR - 2
        assert 0 <= jlo <= jhi <= W
        prm.append(dict(wf=wf, wa=wa, wb=wb, rb=rb, cb=cb, jlo=jlo, jhi=jhi,
                        rows=rows, cols=cols, scale=m * m))
    return prm


def _band(pairs):
    """lhsT[k, m] = sum of w*d(k==m+o) over (o, w) pairs (within block)."""
    mat = np.zeros((P, P), np.float16)
    for o, w in pairs:
        if o >= P:
            continue
        idx = np.arange(P - o)
        mat[idx + o, idx] = np.float16(w)
    return mat


def _corner(pairs):
    """cross-seam lhsT[k, m] = w*d(k==m+o-P) for m >= P-o."""
    mat = np.zeros((P, P), np.float16)
    for o, w in pairs:
        if o <= 0:
            continue
        ms = np.arange(P - o, P)
        mat[ms + o - P, ms] = np.float16(w)
    return mat


def kernel(grid, gt_sym_axis, gd_sym_axis):
    grid = np.ascontiguousarray(grid, dtype=np.float32)
    B = grid.shape[0]
    assert grid.shape == (B, 2, H, W) and B == NS * NCORES

    if "nc" not in _CACHE:
        nc = _build_program()
        _split_multiwaits(nc)
        _CACHE["nc"] = nc
    nc = _CACHE["nc"]

    prm = _host_params(np.asarray(gt_sym_axis), np.asarray(gd_sym_axis))

    i_of_pq = np.arange(H).reshape(Q, P).T
    in_maps = []
    for c in range(NCORES):
        pfv = np.zeros((P, PFW), np.float32)
        fmv = np.zeros((P, FM), np.float16)
        xmv = np.zeros((KX, XW), np.float16)
        for s in range(NS):
            p = prm[c * NS + s]
            b = s & 1
            pfv[:, s * NPF:(s + 1) * NPF] = p["wf"][None, :]
            off = np.array([b * GF + PADL + p["cb"]], np.int32)
            pfv[0, NS * NPF + s] = off.view(np.float32)[0]
            if s == NS - 1:
                po = np.array([b * GF + PADL + p["cb"],
                               b * GF + PADL + p["cb"] + 1], np.int32)
                pfv[0, NS * NPF + NS:NS * NPF + NS + 2] = po.view(np.float32)
                wa0_, wa1_ = p["wa"]
                wb0_, wb1_ = p["wb"]
                rb_ = p["rb"]
                cb0_ = float(p["wf"][2])
                cb1_ = float(p["wf"][3])
                base = NMAT + NIV
                fmv[:, base + 0 * P:base + 1 * P] = _band(
                    [(rb_, wb0_ * cb0_), (rb_ + 1, wb1_ * cb0_)])
                fmv[:, base + 1 * P:base + 2 * P] = _band(
                    [(rb_, wb0_ * cb1_), (rb_ + 1, wb1_ * cb1_)])
                xmv[:, NMAT + 0 * P:NMAT + 1 * P] = _corner(
                    [(rb_, wb0_ * cb0_), (rb_ + 1, wb1_ * cb0_)])[:KX]
                xmv[:, NMAT + 1 * P:NMAT + 2 * P] = _corner(
                    [(rb_, wb0_ * cb1_), (rb_ + 1, wb1_ * cb1_)])[:KX]
            fmv[:, NMAT + s * Q:NMAT + (s + 1) * Q] = (
                i_of_pq < p["rows"]).astype(np.float16)
            wa0, wa1 = p["wa"]
            wb0, wb1 = p["wb"]
            rb = p["rb"]
            fmv[:, (2 * s) * P:(2 * s + 1) * P] = _band(
                [(0, wa0), (1, wa1)])
            fmv[:, (2 * s + 1) * P:(2 * s + 2) * P] = _band(
                [(rb, wb0), (rb + 1, wb1)])
            xmv[:, (2 * s) * P:(2 * s + 1) * P] = _corner(
                [(1, wa1)])[:KX]
            xmv[:, (2 * s + 1) * P:(2 * s + 2) * P] = _corner(
                [(rb, wb0), (rb + 1, wb1)])[:KX]
        in_maps.append({
            "g": grid[c * NS:(c + 1) * NS], "pfi": pfv, "fmats": fmv,
            "xm": xmv,
        })

    res = run_bass_kernel_spmd(nc, in_maps, core_ids=list(range(NCORES)))

    losses = np.zeros(B, np.float64)
    for c in range(NCORES):
        o = res.results[c]["out"]
        for s in range(NS):
            p = prm[c * NS + s]
            ssq = float(o[s, p["jlo"]:p["jhi"]].sum(dtype=np.float64))
            count = float(np.float32(p["rows"] * p["cols"]))
            losses[c * NS + s] = p["scale"] * ssq / count
    return np.float32(losses.mean())
